# revision 6
# baseline (speedup 1.0000x reference)
"""8-bit ripple-carry adder on 8 TRN2 NeuronCores.

Full inputs A[N,8], B[N,8] (MSB-first bits in {0,1} as f32), Cin[N,1].
Returns (out[N,8], carry[N,1]) matching the reference.

Strategy: pure data-parallel over the batch dim (N/8 rows per core).
Per core, rows are tiled [128 partitions x F rows]. Per row the carry
recurrence is c' = (a + b + c >= 2); with t = a + b laid out in 10-wide
groups [b7..b0, slot, slot] (slot = 2*Cin, which forces the next state to
Cin and so resets the chain between rows), a single reversed DVE
tensor_tensor_scan  state = (t + state) >= 2  computes every carry of
every row. Sum bits are (t == 1) XOR carry_in.

bf16 is used for all DVE elementwise ops (values stay in {0,1,2} -
exact), keeping them in the DVE's 2x/4x perf modes; f32<->bf16 converts
run on the Scalar engine, which has its own SBUF ports. GPSIMD is
deliberately unused (it shares SBUF ports with the DVE; concurrent
GPSIMD ops slow DVE tensor ops ~3x, measured). Deep tile-pool buffering
on the input side keeps the DMA stream running ahead of compute.
"""

import sys

import numpy as np

if "/opt/trn_rl_repo" not in sys.path:
    sys.path.insert(0, "/opt/trn_rl_repo")

N_BITS = 8
P = 128
N_CORES = 8


def build_adder_nc(R: int, F: int):
    """Per-core Bass program for an R-row shard, F rows/partition/tile."""
    import concourse.bacc as bacc
    import concourse.mybir as mybir
    from concourse.mybir import AluOpType
    from concourse.tile import TileContext

    f32 = mybir.dt.float32
    bf16 = mybir.dt.bfloat16
    Copy = mybir.ActivationFunctionType.Copy
    G = 10  # group width: 8 bits MSB-first + 2 reset slots
    W = G * F
    rows_per_tile = P * F
    assert R % rows_per_tile == 0
    T = R // rows_per_tile

    nc = bacc.Bacc("TRN2", target_bir_lowering=False, debug=False)

    A = nc.declare_dram_parameter("A", [R, N_BITS], f32, isOutput=False)
    B = nc.declare_dram_parameter("B", [R, N_BITS], f32, isOutput=False)
    CIN = nc.declare_dram_parameter("Cin", [R, 1], f32, isOutput=False)
    OUT = nc.declare_dram_parameter("out", [R, N_BITS], f32, isOutput=True)
    COUT = nc.declare_dram_parameter("cout", [R, 1], f32, isOutput=True)

    # [T, 128, F*8] / [T, 128, F]; partition p owns F contiguous DRAM rows.
    A_t = A[:].rearrange("(t p f) b -> t p (f b)", p=P, f=F)
    B_t = B[:].rearrange("(t p f) b -> t p (f b)", p=P, f=F)
    C_t = CIN[:].rearrange("(t p f) b -> t p (f b)", p=P, f=F)
    O_t = OUT[:].rearrange("(t p f) b -> t p (f b)", p=P, f=F)
    K_t = COUT[:].rearrange("(t p f) b -> t p (f b)", p=P, f=F)

    with TileContext(nc) as tc:
        with (
            tc.tile_pool(name="const", bufs=1) as const_pool,
            tc.tile_pool(name="io", bufs=4) as io_pool,
            tc.tile_pool(name="cin", bufs=6) as cin_pool,
            tc.tile_pool(name="work", bufs=3) as work_pool,
            tc.tile_pool(name="outp", bufs=3) as out_pool,
        ):
            two = const_pool.tile([P, W], bf16)
            nc.vector.memset(two[:], 2.0)

            tiles = {}

            def loads(i):
                a32 = io_pool.tile([P, N_BITS * F], f32, tag="a32")
                b32 = io_pool.tile([P, N_BITS * F], f32, tag="b32")
                c = cin_pool.tile([P, F], f32, tag="c")
                nc.sync.dma_start(out=a32[:], in_=A_t[i])
                nc.sync.dma_start(out=b32[:], in_=B_t[i])
                nc.sync.dma_start(out=c[:], in_=C_t[i])
                tiles[i] = (a32, b32, c)

            def converts(i):
                a32, b32, c = tiles[i]
                abf = work_pool.tile([P, N_BITS * F], bf16, tag="abf")
                bbf = work_pool.tile([P, N_BITS * F], bf16, tag="bbf")
                t = work_pool.tile([P, W], bf16, tag="t")
                nc.scalar.copy(out=abf[:], in_=a32[:])
                nc.scalar.copy(out=bbf[:], in_=b32[:])
                # slots: 2*Cin (chain reset)
                t3 = t[:].rearrange("p (f n) -> p f n", n=G)
                nc.scalar.activation(
                    out=t3[:, :, 8:10],
                    in_=c[:].unsqueeze(2).broadcast_to([P, F, 2]),
                    func=Copy,
                    scale=2.0,
                )
                tiles[i] = (abf, bbf, t)

            def compute(i):
                abf, bbf, t = tiles.pop(i)
                sc = work_pool.tile([P, W + 1], bf16, tag="sc")
                px = work_pool.tile([P, N_BITS * F], bf16, tag="px")
                sbf = work_pool.tile([P, N_BITS * F], bf16, tag="sbf")
                s32 = out_pool.tile([P, N_BITS * F], f32, tag="s32")
                k = out_pool.tile([P, F], f32, tag="k")

                t3 = t[:].rearrange("p (f n) -> p f n", n=G)
                a3 = abf[:].rearrange("p (f n) -> p f n", n=N_BITS)
                b3 = bbf[:].rearrange("p (f n) -> p f n", n=N_BITS)

                # bits: t = a + b in {0,1,2}
                nc.vector.tensor_tensor(t3[:, :, 0:8], a3, b3, AluOpType.add)

                # reversed scan, output shifted by one: sc[q+1] = state after q
                # state = (t[q] + state) >= 2 -> every carry, LSB->MSB per row
                nc.vector.tensor_tensor_scan(
                    sc[:, 1 : W + 1][:, ::-1],
                    t[:][:, ::-1],
                    two[:][:, ::-1],
                    0.0,
                    AluOpType.add,
                    AluOpType.is_ge,
                )

                # p = (t == 1) = a XOR b ; s = p XOR carry_in
                nc.vector.tensor_scalar(
                    px[:], t3[:, :, 0:8], 1.0, None, AluOpType.is_equal
                )
                sc3 = sc[:, 0:W].rearrange("p (f n) -> p f n", n=G)
                nc.vector.tensor_tensor(
                    sbf[:],
                    px[:].rearrange("p (f n) -> p f n", n=N_BITS),
                    sc3[:, :, 2:10],
                    AluOpType.not_equal,
                )

                # carry-out of row f = state after its MSB = sc[10f+1]
                sc_k = sc[:, 1 : W + 1].rearrange("p (f n) -> p f n", n=G)
                nc.scalar.copy(out=k[:].unsqueeze(2), in_=sc_k[:, :, 0:1])
                nc.scalar.copy(out=s32[:], in_=sbf[:])

                nc.sync.dma_start(out=O_t[i], in_=s32[:])
                nc.sync.dma_start(out=K_t[i], in_=k[:])

            # software pipeline: loads run 2 ahead, converts 1 ahead, so
            # iteration i+1's ACT converts are queued before iteration i's
            # output ops and the ACT FIFO never carries the loop dependency
            loads(0)
            if T > 1:
                loads(1)
            converts(0)
            for i in range(T):
                if i + 2 < T:
                    loads(i + 2)
                if i + 1 < T:
                    converts(i + 1)
                compute(i)

    nc.compile()
    return nc


def _run(nc, in_maps, trace=False):
    from concourse.bass_utils import run_bass_kernel_spmd

    return run_bass_kernel_spmd(
        nc, in_maps, core_ids=list(range(N_CORES)), trace=trace
    )


def kernel(A: np.ndarray, B: np.ndarray, Cin: np.ndarray):
    N = A.shape[0]
    R = N // N_CORES
    A = np.ascontiguousarray(A, dtype=np.float32)
    B = np.ascontiguousarray(B, dtype=np.float32)
    Cin = np.ascontiguousarray(Cin, dtype=np.float32)

    nc = build_adder_nc(R, F=256)
    in_maps = [
        {
            "A": A[i * R : (i + 1) * R],
            "B": B[i * R : (i + 1) * R],
            "Cin": Cin[i * R : (i + 1) * R],
        }
        for i in range(N_CORES)
    ]
    res = _run(nc, in_maps)
    out = np.concatenate([res.results[i]["out"] for i in range(N_CORES)], axis=0)
    cout = np.concatenate([res.results[i]["cout"] for i in range(N_CORES)], axis=0)
    return out, cout


# revision 7
# speedup vs baseline: 1.1389x; 1.1389x over previous
"""8-bit ripple-carry adder on 8 TRN2 NeuronCores.

Full inputs A[N,8], B[N,8] (MSB-first bits in {0,1} as f32), Cin[N,1].
Returns (out[N,8], carry[N,1]) matching the reference.

Strategy: pure data-parallel over the batch dim (N/8 rows per core).
Per core, rows are tiled [128 partitions x F rows]. Per row the carry
recurrence is c' = (a + b + c >= 2); with t = a + b laid out in 10-wide
groups [b7..b0, slot, slot] (slot = 2*Cin, which forces the next state to
Cin and so resets the chain between rows), a single reversed DVE
tensor_tensor_scan  state = (t + state) >= 2  computes every carry of
every row. Sum bits are (t == 1) XOR carry_in.

bf16 is used for all DVE elementwise ops (values stay in {0,1,2} -
exact), keeping them in the DVE's 2x/4x perf modes; f32<->bf16 converts
run on the Scalar engine, which has its own SBUF ports. GPSIMD is
deliberately unused (it shares SBUF ports with the DVE; concurrent
GPSIMD ops slow DVE tensor ops ~3x, measured). Deep tile-pool buffering
on the input side keeps the DMA stream running ahead of compute.
"""

import sys

import numpy as np

if "/opt/trn_rl_repo" not in sys.path:
    sys.path.insert(0, "/opt/trn_rl_repo")

N_BITS = 8
P = 128
N_CORES = 8


def build_adder_nc(R: int, F: int):
    """Per-core Bass program for an R-row shard, F rows/partition/tile."""
    import concourse.bacc as bacc
    import concourse.mybir as mybir
    from concourse.mybir import AluOpType
    from concourse.tile import TileContext

    f32 = mybir.dt.float32
    bf16 = mybir.dt.bfloat16
    Copy = mybir.ActivationFunctionType.Copy
    G = 10  # group width: 8 bits MSB-first + 2 reset slots
    W = G * F
    rows_per_tile = P * F
    assert R % rows_per_tile == 0
    T = R // rows_per_tile

    nc = bacc.Bacc("TRN2", target_bir_lowering=False, debug=False)

    A = nc.declare_dram_parameter("A", [R, N_BITS], f32, isOutput=False)
    B = nc.declare_dram_parameter("B", [R, N_BITS], f32, isOutput=False)
    CIN = nc.declare_dram_parameter("Cin", [R, 1], f32, isOutput=False)
    OUT = nc.declare_dram_parameter("out", [R, N_BITS], f32, isOutput=True)
    COUT = nc.declare_dram_parameter("cout", [R, 1], f32, isOutput=True)

    # [T, 128, F*8] / [T, 128, F]; partition p owns F contiguous DRAM rows.
    A_t = A[:].rearrange("(t p f) b -> t p (f b)", p=P, f=F)
    B_t = B[:].rearrange("(t p f) b -> t p (f b)", p=P, f=F)
    C_t = CIN[:].rearrange("(t p f) b -> t p (f b)", p=P, f=F)
    O_t = OUT[:].rearrange("(t p f) b -> t p (f b)", p=P, f=F)
    K_t = COUT[:].rearrange("(t p f) b -> t p (f b)", p=P, f=F)

    with TileContext(nc) as tc:
        with (
            tc.tile_pool(name="const", bufs=1) as const_pool,
            tc.tile_pool(name="io", bufs=4) as io_pool,
            tc.tile_pool(name="cin", bufs=6) as cin_pool,
            tc.tile_pool(name="work", bufs=3) as work_pool,
            tc.tile_pool(name="outp", bufs=3) as out_pool,
        ):
            two = const_pool.tile([P, W], bf16)
            nc.vector.memset(two[:], 2.0)

            tiles = {}

            def loads(i):
                a32 = io_pool.tile([P, N_BITS * F], f32, tag="a32")
                b32 = io_pool.tile([P, N_BITS * F], f32, tag="b32")
                c = cin_pool.tile([P, F], f32, tag="c")
                nc.sync.dma_start(out=a32[:], in_=A_t[i])
                nc.sync.dma_start(out=b32[:], in_=B_t[i])
                nc.sync.dma_start(out=c[:], in_=C_t[i])
                tiles[i] = (a32, b32, c)

            def converts(i):
                a32, b32, c = tiles[i]
                abf = work_pool.tile([P, N_BITS * F], bf16, tag="abf")
                bbf = work_pool.tile([P, N_BITS * F], bf16, tag="bbf")
                t = work_pool.tile([P, W], bf16, tag="t")
                nc.scalar.copy(out=abf[:], in_=a32[:])
                nc.scalar.copy(out=bbf[:], in_=b32[:])
                # slots: 2*Cin (chain reset)
                t3 = t[:].rearrange("p (f n) -> p f n", n=G)
                nc.scalar.activation(
                    out=t3[:, :, 8:10],
                    in_=c[:].unsqueeze(2).broadcast_to([P, F, 2]),
                    func=Copy,
                    scale=2.0,
                )
                tiles[i] = (abf, bbf, t)

            def compute(i):
                abf, bbf, t = tiles.pop(i)
                sc = work_pool.tile([P, W + 1], bf16, tag="sc")
                s32 = out_pool.tile([P, N_BITS * F], f32, tag="s32")
                k = out_pool.tile([P, F], f32, tag="k")

                t3 = t[:].rearrange("p (f n) -> p f n", n=G)
                a3 = abf[:].rearrange("p (f n) -> p f n", n=N_BITS)
                b3 = bbf[:].rearrange("p (f n) -> p f n", n=N_BITS)

                # bits: t = a + b in {0,1,2}
                nc.vector.tensor_tensor(t3[:, :, 0:8], a3, b3, AluOpType.add)

                # reversed scan, output shifted by one: sc[q+1] = state after q
                # state = (t[q] + state) >= 2 -> every carry, LSB->MSB per row
                nc.vector.tensor_tensor_scan(
                    sc[:, 1 : W + 1][:, ::-1],
                    t[:][:, ::-1],
                    two[:][:, ::-1],
                    0.0,
                    AluOpType.add,
                    AluOpType.is_ge,
                )

                # s = (t == 1) XOR carry_in, written as f32 in one fused op
                sc3 = sc[:, 0:W].rearrange("p (f n) -> p f n", n=G)
                nc.vector.scalar_tensor_tensor(
                    out=s32[:].rearrange("p (f n) -> p f n", n=N_BITS),
                    in0=t3[:, :, 0:8],
                    scalar=1.0,
                    in1=sc3[:, :, 2:10],
                    op0=AluOpType.is_equal,
                    op1=AluOpType.not_equal,
                )

                # carry-out of row f = state after its MSB = sc[10f+1]
                sc_k = sc[:, 1 : W + 1].rearrange("p (f n) -> p f n", n=G)
                nc.scalar.copy(out=k[:].unsqueeze(2), in_=sc_k[:, :, 0:1])

                nc.sync.dma_start(out=O_t[i], in_=s32[:])
                nc.sync.dma_start(out=K_t[i], in_=k[:])

            # software pipeline: loads run 3 ahead, converts 1 ahead, so the
            # ACT FIFO only carries upstream work and the DVE never waits on
            # a cross-engine loop dependency
            loads(0)
            if T > 1:
                loads(1)
            if T > 2:
                loads(2)
            converts(0)
            for i in range(T):
                if i + 3 < T:
                    loads(i + 3)
                if i + 1 < T:
                    converts(i + 1)
                compute(i)

    nc.compile()
    return nc


def _run(nc, in_maps, trace=False):
    from concourse.bass_utils import run_bass_kernel_spmd

    return run_bass_kernel_spmd(
        nc, in_maps, core_ids=list(range(N_CORES)), trace=trace
    )


def kernel(A: np.ndarray, B: np.ndarray, Cin: np.ndarray):
    N = A.shape[0]
    R = N // N_CORES
    A = np.ascontiguousarray(A, dtype=np.float32)
    B = np.ascontiguousarray(B, dtype=np.float32)
    Cin = np.ascontiguousarray(Cin, dtype=np.float32)

    nc = build_adder_nc(R, F=256)
    in_maps = [
        {
            "A": A[i * R : (i + 1) * R],
            "B": B[i * R : (i + 1) * R],
            "Cin": Cin[i * R : (i + 1) * R],
        }
        for i in range(N_CORES)
    ]
    res = _run(nc, in_maps)
    out = np.concatenate([res.results[i]["out"] for i in range(N_CORES)], axis=0)
    cout = np.concatenate([res.results[i]["cout"] for i in range(N_CORES)], axis=0)
    return out, cout


# revision 11
# speedup vs baseline: 1.3428x; 1.1790x over previous
"""8-bit ripple-carry adder on 8 TRN2 NeuronCores.

Full inputs A[N,8], B[N,8] (MSB-first bits in {0,1} as f32), Cin[N,1].
Returns (out[N,8], carry[N,1]) matching the reference.

Strategy: pure data-parallel over the batch dim (N/8 rows per core).
Per core, rows are tiled [128 partitions x F rows]. Per row the carry
recurrence is c' = (a + b + c >= 2); with t = a + b laid out in 10-wide
groups [b7..b0, slot, slot] (slot = 2*Cin, which forces the next state to
Cin and so resets the chain between rows), a single reversed DVE
tensor_tensor_scan  state = (t + state) >= 2  computes every carry of
every row. Sum bits are (t == 1) XOR carry_in.

bf16 is used for all DVE elementwise ops (values stay in {0,1,2} -
exact), keeping them in the DVE's 2x/4x perf modes; f32<->bf16 converts
run on the Scalar engine, which has its own SBUF ports. GPSIMD is
deliberately unused (it shares SBUF ports with the DVE; concurrent
GPSIMD ops slow DVE tensor ops ~3x, measured). Deep tile-pool buffering
on the input side keeps the DMA stream running ahead of compute.
"""

import sys

import numpy as np

if "/opt/trn_rl_repo" not in sys.path:
    sys.path.insert(0, "/opt/trn_rl_repo")

N_BITS = 8
P = 128
N_CORES = 8


def build_adder_nc(R: int, F: int):
    """Per-core Bass program for an R-row shard, F rows/partition/tile."""
    import concourse.bacc as bacc
    import concourse.mybir as mybir
    from concourse.mybir import AluOpType
    from concourse.tile import TileContext, add_dep_helper

    f32 = mybir.dt.float32
    bf16 = mybir.dt.bfloat16
    Copy = mybir.ActivationFunctionType.Copy
    G = 10  # group width: 8 bits MSB-first + 2 reset slots
    W = G * F
    rows_per_tile = P * F
    assert R % rows_per_tile == 0
    T = R // rows_per_tile

    nc = bacc.Bacc("TRN2", target_bir_lowering=False, debug=False)

    A = nc.declare_dram_parameter("A", [R, N_BITS], f32, isOutput=False)
    B = nc.declare_dram_parameter("B", [R, N_BITS], f32, isOutput=False)
    CIN = nc.declare_dram_parameter("Cin", [R, 1], f32, isOutput=False)
    OUT = nc.declare_dram_parameter("out", [R, N_BITS], f32, isOutput=True)
    COUT = nc.declare_dram_parameter("cout", [R, 1], f32, isOutput=True)

    # [T, 128, F*8] / [T, 128, F]; partition p owns F contiguous DRAM rows.
    A_t = A[:].rearrange("(t p f) b -> t p (f b)", p=P, f=F)
    B_t = B[:].rearrange("(t p f) b -> t p (f b)", p=P, f=F)
    C_t = CIN[:].rearrange("(t p f) b -> t p (f b)", p=P, f=F)
    O_t = OUT[:].rearrange("(t p f) b -> t p (f b)", p=P, f=F)
    K_t = COUT[:].rearrange("(t p f) b -> t p (f b)", p=P, f=F)

    with TileContext(nc) as tc:
        with (
            tc.tile_pool(name="const", bufs=1) as const_pool,
            tc.tile_pool(name="io", bufs=4) as io_pool,
            tc.tile_pool(name="cin", bufs=6) as cin_pool,
            tc.tile_pool(name="work", bufs=3) as work_pool,
            tc.tile_pool(name="outp", bufs=3) as out_pool,
        ):
            two = const_pool.tile([P, W], bf16)
            nc.vector.memset(two[:], 2.0)

            tiles = {}
            slot_insts = {}

            def loads(i):
                a32 = io_pool.tile([P, N_BITS * F], f32, tag="a32")
                b32 = io_pool.tile([P, N_BITS * F], f32, tag="b32")
                c = cin_pool.tile([P, F], f32, tag="c")
                nc.sync.dma_start(out=a32[:], in_=A_t[i])
                nc.sync.dma_start(out=b32[:], in_=B_t[i])
                nc.sync.dma_start(out=c[:], in_=C_t[i])
                tiles[i] = (a32, b32, c)

            def converts(i):
                a32, b32, c = tiles[i]
                abf = work_pool.tile([P, N_BITS * F], bf16, tag="abf")
                bbf = work_pool.tile([P, N_BITS * F], bf16, tag="bbf")
                t = work_pool.tile([P, W], bf16, tag="t")
                nc.scalar.copy(out=abf[:], in_=a32[:])
                nc.scalar.copy(out=bbf[:], in_=b32[:])
                # slots: 2*Cin (chain reset)
                t3 = t[:].rearrange("p (f n) -> p f n", n=G)
                slot_inst = nc.scalar.activation(
                    out=t3[:, :, 8:10],
                    in_=c[:].unsqueeze(2).broadcast_to([P, F, 2]),
                    func=Copy,
                    scale=2.0,
                )
                slot_insts[i] = slot_inst.ins
                tiles[i] = (abf, bbf, t)

            def compute(i):
                abf, bbf, t = tiles.pop(i)
                sc = work_pool.tile([P, W + 1], bf16, tag="sc")
                s32 = out_pool.tile([P, N_BITS * F], f32, tag="s32")
                k = out_pool.tile([P, F], f32, tag="k")

                t3 = t[:].rearrange("p (f n) -> p f n", n=G)
                a3 = abf[:].rearrange("p (f n) -> p f n", n=N_BITS)
                b3 = bbf[:].rearrange("p (f n) -> p f n", n=N_BITS)

                # bits: t = a + b in {0,1,2}
                nc.vector.tensor_tensor(t3[:, :, 0:8], a3, b3, AluOpType.add)

                # reversed scan, output shifted by one: sc[q+1] = state after q
                # state = (t[q] + state) >= 2 -> every carry, LSB->MSB per row
                nc.vector.tensor_tensor_scan(
                    sc[:, 1 : W + 1][:, ::-1],
                    t[:][:, ::-1],
                    two[:][:, ::-1],
                    0.0,
                    AluOpType.add,
                    AluOpType.is_ge,
                )

                # s = (t == 1) XOR carry_in, written as f32 in one fused op
                sc3 = sc[:, 0:W].rearrange("p (f n) -> p f n", n=G)
                nc.vector.scalar_tensor_tensor(
                    out=s32[:].rearrange("p (f n) -> p f n", n=N_BITS),
                    in0=t3[:, :, 0:8],
                    scalar=1.0,
                    in1=sc3[:, :, 2:10],
                    op0=AluOpType.is_equal,
                    op1=AluOpType.not_equal,
                )

                # carry-out of row f = state after its MSB = sc[10f+1]
                sc_k = sc[:, 1 : W + 1].rearrange("p (f n) -> p f n", n=G)
                k_inst = nc.scalar.copy(out=k[:].unsqueeze(2), in_=sc_k[:, :, 0:1])
                # keep this (scan-gated) copy BEHIND the next tile's upstream
                # ACT work in the engine FIFO, or it stalls those converts
                if i + 1 in slot_insts:
                    add_dep_helper(
                        k_inst.ins,
                        slot_insts[i + 1],
                        sync=False,
                        reason="ACT upstream-before-downstream",
                    )

                nc.sync.dma_start(out=O_t[i], in_=s32[:])
                nc.sync.dma_start(out=K_t[i], in_=k[:])

            # software pipeline: loads run 3 ahead, converts 1 ahead, so the
            # ACT FIFO only carries upstream work and the DVE never waits on
            # a cross-engine loop dependency
            loads(0)
            if T > 1:
                loads(1)
            if T > 2:
                loads(2)
            converts(0)
            for i in range(T):
                if i + 3 < T:
                    loads(i + 3)
                if i + 1 < T:
                    converts(i + 1)
                compute(i)

    nc.compile()
    return nc


def _run(nc, in_maps, trace=False):
    from concourse.bass_utils import run_bass_kernel_spmd

    return run_bass_kernel_spmd(
        nc, in_maps, core_ids=list(range(N_CORES)), trace=trace
    )


def kernel(A: np.ndarray, B: np.ndarray, Cin: np.ndarray):
    N = A.shape[0]
    R = N // N_CORES
    A = np.ascontiguousarray(A, dtype=np.float32)
    B = np.ascontiguousarray(B, dtype=np.float32)
    Cin = np.ascontiguousarray(Cin, dtype=np.float32)

    nc = build_adder_nc(R, F=256)
    in_maps = [
        {
            "A": A[i * R : (i + 1) * R],
            "B": B[i * R : (i + 1) * R],
            "Cin": Cin[i * R : (i + 1) * R],
        }
        for i in range(N_CORES)
    ]
    res = _run(nc, in_maps)
    out = np.concatenate([res.results[i]["out"] for i in range(N_CORES)], axis=0)
    cout = np.concatenate([res.results[i]["cout"] for i in range(N_CORES)], axis=0)
    return out, cout
